# revision 1
# baseline (speedup 1.0000x reference)
"""Trainium2 Bass kernel for nn_AdvancedRegressionModel (20-qubit circuit regression).

Math: the reference circuit collapses to out_b = sum_j g_j |(A psi_b)_j|^2 + b0
where A = kron of 20 fused 2x2 gates (RY_k @ RX_k per wire) and g absorbs the
CNOT chain (a basis permutation), the <Z_i> measurements, and the linear head
via prefix-sign weights: g(a,m,z) = ga(a) + sA(a)*gm(m) + sA(a)*sM(m)*gz(z).

Wire blocks: q = wires 0-6 (a:0-5 + m1:6), m2 = wires 7-12, z = wires 13-19.
Per batch row (4 rows/core, batch-sharded over 8 cores):
  load  [q:128, (m2:64, z:128)] - plain contiguous chunks          (DMA)
  P0    contract q: stat-data matmuls per m2 tile -> [z, q'_re|im] (PE)
  P1'   contract z: stat-data matmuls per stride-64 column pick t
        -> partitions p2 = 2*m2 + h  (h = top bit of q')           (PE)
  P2'   apply kron(Um2, I2) on p2 (normal matmuls, f32r)           (PE)
  SQ    squares of re/im PSUM -> SBUF                              (ACT)
  P4    reduce-matmul with 16-col sign-weight stationary R
        -> F[10, (t:64, z':128)] per row                           (PE)
  host  finish: out_r = sum_kn F[k,n] * w_k(n) + b

f32r (tf32-like) keeps rel err ~2.6e-3.  Self-loading f32r matmuls allow only
ONE sync-wait, so cross-engine dependency frontiers are funneled through tiny
"touch" matmuls; PSUM drains split DVE/ACT ~2:1 by group for engine balance.
"""

import numpy as np
from contextlib import ExitStack

import concourse.mybir as mybir
from concourse import bacc, bass, tile
from concourse.bass_utils import run_bass_kernel_spmd
from concourse.tile_rust import add_dep_helper

NW = 20
DIM = 2 ** NW
BATCH = 32
NCORES = 8
RPC = BATCH // NCORES  # rows per core

F32 = mybir.dt.float32
F32R = mybir.dt.float32r
BF16 = mybir.dt.bfloat16
# bf16 data path exists but is disabled: FWL requires contiguous weights and
# the P1' stationaries are stride-64 picks (NaN on HW); measured precision
# ladder also puts full-bf16 at ~2e-2 rel err. f32r is the shipped config.
USE_BF16 = False
DT = F32R


# ----------------------------------------------------------------- host math
def _gates(params):
    A = []
    for k in range(NW):
        c, s = np.cos(params[k] / 2), np.sin(params[k] / 2)
        RX = np.array([[c, -1j * s], [-1j * s, c]], dtype=np.complex128)
        c2, s2 = np.cos(params[k + NW] / 2), np.sin(params[k + NW] / 2)
        RY = np.array([[c2, -s2], [s2, c2]], dtype=np.complex128)
        A.append(RY @ RX)
    return A


def _kron_list(lst):
    out = lst[0]
    for x in lst[1:]:
        out = np.kron(out, x)
    return out


def _prefix_weights(Wv, wire_lo, wire_hi):
    n = wire_hi - wire_lo + 1
    v = np.arange(2 ** n)
    signs = np.stack([1 - 2 * ((v >> (n - 1 - k)) & 1) for k in range(n)])
    pref = np.cumprod(signs, axis=0)
    g = sum(Wv[wire_lo + i] * pref[i] for i in range(n))
    return g.astype(np.float64), pref[n - 1].astype(np.float64)


def _host_precompute(params, W):
    params = np.asarray(params, dtype=np.float64)
    Wv = np.asarray(W, dtype=np.float64).reshape(-1)
    A = _gates(params)
    Ua = _kron_list(A[0:6])        # 64x64
    Gq = np.kron(Ua, A[6])         # 128x128, contracts q = (a, m1)
    Uz = _kron_list(A[13:20])      # 128x128, contracts z
    Um2 = _kron_list(A[7:13])      # 64x64
    G3 = np.kron(Um2, np.eye(2))   # 128x128 on p2 = (m2, h)

    G1 = np.concatenate([Gq.real.T, Gq.imag.T], axis=1)
    G2a = np.concatenate([Uz.real.T, Uz.imag.T], axis=1)
    G2b = np.concatenate([-Uz.imag.T, Uz.real.T], axis=1)
    G3S = np.concatenate([G3.real.T, G3.imag.T, -G3.imag.T], axis=1)

    ga, sA = _prefix_weights(Wv, 0, 5)    # [64]
    gm, sM = _prefix_weights(Wv, 6, 12)   # [128]
    gz, _ = _prefix_weights(Wv, 13, 19)  # [128]

    # R' [128, 10]: partition p = m2'*2 + h
    p = np.arange(128)
    m2p, h = p >> 1, p & 1
    cols = [(h == 0).astype(float), (h == 1).astype(float)]
    for h0 in (0, 1):
        for m10 in (0, 1):
            cols.append((h == h0) * gm[m10 * 64 + m2p])
    for h0 in (0, 1):
        for m10 in (0, 1):
            cols.append((h == h0) * sM[m10 * 64 + m2p])
    R = np.stack(cols, axis=1)
    R = np.concatenate([R, np.zeros((128, 6))], axis=1)  # pad to 16 cols

    # free weights w[k, n], n = t*128 + z'; a' = h*32 + (t>>1), m1' = t&1
    n = np.arange(8192)
    t, zp = n >> 7, n & 127
    m1p = t & 1
    w = np.zeros((10, 8192))
    w[0] = ga[0 * 32 + (t >> 1)]
    w[1] = ga[1 * 32 + (t >> 1)]
    k = 2
    for h0 in (0, 1):
        for m10 in (0, 1):
            w[k] = (m1p == m10) * sA[h0 * 32 + (t >> 1)]
            k += 1
    for h0 in (0, 1):
        for m10 in (0, 1):
            w[k] = (m1p == m10) * sA[h0 * 32 + (t >> 1)] * gz[zp]
            k += 1

    import ml_dtypes
    cdt = ml_dtypes.bfloat16 if USE_BF16 else np.float32
    consts = {
        "G1": np.ascontiguousarray(G1, dtype=cdt),
        "G2a": np.ascontiguousarray(G2a, dtype=cdt),
        "G2b": np.ascontiguousarray(G2b, dtype=cdt),
        "G3S": np.ascontiguousarray(G3S, dtype=cdt),
        "R": np.ascontiguousarray(R, dtype=np.float32),
    }
    return consts, w


def _host_finish(F, w, b):
    # F: [B, 10, 8192]
    out = np.einsum("bkn,kn->b", F.astype(np.float64), w)
    return out + float(np.asarray(b).reshape(-1)[0])


# ------------------------------------------------------------- bass program
def build_bass():
    nc = bacc.Bacc("TRN2", target_bir_lowering=False)
    st = nc.declare_dram_parameter("state", [RPC, DIM], DT, isOutput=False)
    g1 = nc.declare_dram_parameter("G1", [128, 256], DT, isOutput=False)
    g2a = nc.declare_dram_parameter("G2a", [128, 256], DT, isOutput=False)
    g2b = nc.declare_dram_parameter("G2b", [128, 256], DT, isOutput=False)
    g3s = nc.declare_dram_parameter("G3S", [128, 384], DT, isOutput=False)
    rw = nc.declare_dram_parameter("R", [128, 16], F32R, isOutput=False)
    fout = nc.declare_dram_parameter("F", [RPC, 10, 8192], F32, isOutput=True)
    dbg = nc.declare_dram_parameter("dbg", [1, 1], F32, isOutput=True)

    with ExitStack() as ctx:
        tc = ctx.enter_context(tile.TileContext(nc))
        const_pool = ctx.enter_context(tc.tile_pool(name="const", bufs=1))
        x0_pool = ctx.enter_context(tc.tile_pool(name="x0", bufs=2))
        y_pool = ctx.enter_context(tc.tile_pool(name="y", bufs=1))
        l2_pool = ctx.enter_context(tc.tile_pool(name="l2", bufs=3))
        sq_pool = ctx.enter_context(tc.tile_pool(name="sq", bufs=3))
        f_pool = ctx.enter_context(tc.tile_pool(name="f", bufs=3))
        psA_pool = ctx.enter_context(
            tc.tile_pool(name="psA", bufs=2, space=bass.MemorySpace.PSUM))
        psB_pool = ctx.enter_context(
            tc.tile_pool(name="psB", bufs=1, space=bass.MemorySpace.PSUM))
        ps4_pool = ctx.enter_context(
            tc.tile_pool(name="ps4", bufs=1, space=bass.MemorySpace.PSUM))
        scr_pool = ctx.enter_context(
            tc.tile_pool(name="scr", bufs=1, space=bass.MemorySpace.PSUM))

        G1 = const_pool.tile([128, 256], DT)
        G2a = const_pool.tile([128, 256], DT)
        G2b = const_pool.tile([128, 256], DT)
        G3S = const_pool.tile([128, 384], DT)
        R = const_pool.tile([128, 16], F32R)
        scratch = scr_pool.tile([128, 16], F32)

        # "touch" = tiny PE matmul whose only job is to make the PE observe
        # one producer-proc's semaphore, so real f32r matmuls (1 wait max)
        # never need multi-proc wait frontiers.
        def touch(ap_128xN):
            # N=2 moving cols: fp32r matmuls need even free dims (8B PSUM lines)
            return nc.tensor.matmul(
                scratch[0:1, 0:2], ap_128xN[:, 0:1], ap_128xN[:, 0:2],
                start=True, stop=True)

        def pin(after, *before):
            for binst in before:
                add_dep_helper(after.ins, binst.ins, False,
                               "touch ordering")

        def load_row(r):
            srcv = st[r].rearrange("(q f) -> q f", q=128, f=8192)
            x0s, x0t = [], []
            for c in range(16):
                xt = x0_pool.tile([128, 512], DT, tag=f"x0{c}")
                nc.sync.dma_start(xt[:], srcv[:, 512 * c:512 * c + 512])
                x0s.append(xt)
                x0t.append(touch(xt[:]))
            return x0s, x0t

        # row 0's 16 chunk DMAs go out first so they own all queues at start
        preload = {0: load_row(0)}

        nc.sync.dma_start(G1[:], g1[:])
        nc.sync.dma_start(G2a[:], g2a[:])
        nc.sync.dma_start(G2b[:], g2b[:])
        nc.sync.dma_start(G3S[:], g3s[:])
        nc.sync.dma_start(R[:], rw[:])
        tG1 = touch(G1[:])
        tG2a = touch(G2a[:])
        tG2b = touch(G2b[:])
        tG3S = touch(G3S[:])
        tR = touch(R[:])
        const_touches = [tG1, tG2a, tG2b, tG3S, tR]

        for r in range(RPC):
            # ---- load [q:128, (m2:64, z:128)] in 16 contiguous chunk DMAs
            x0s, x0_touch = preload[r] if r in preload else load_row(r)

            yre = y_pool.tile([128, 8192], DT, tag="yre")
            yim = y_pool.tile([128, 8192], DT, tag="yim")

            # ---- P0: contract q=(a,m1).  out[z, q'_re|q'_im] per m2.
            for g in range(16):
                ps = psA_pool.tile([128, 1024], F32, tag="ps")
                for q in range(4):
                    a = 4 * g + q
                    xa = x0s[a // 4][:, 128 * (a % 4):128 * (a % 4) + 128]
                    mm = nc.tensor.matmul(
                        ps[:, 256 * q:256 * q + 256], xa, G1[:],
                        start=True, stop=True)
                    if a % 4 == 0:
                        pin(mm, x0_touch[a // 4], *const_touches)
                pv = ps[:].rearrange("p (q t m) -> p q t m", q=4, t=2, m=128)
                yre_v = yre[:, 512 * g:512 * g + 512].rearrange(
                    "p (q m) -> p q m", q=4, m=128)
                yim_v = yim[:, 512 * g:512 * g + 512].rearrange(
                    "p (q m) -> p q m", q=4, m=128)
                if g % 3 == 2:
                    nc.scalar.copy(yre_v, pv[:, :, 0, :])
                    nc.scalar.copy(yim_v, pv[:, :, 1, :])
                else:
                    nc.vector.tensor_copy(yre_v, pv[:, :, 0, :])
                    nc.vector.tensor_copy(yim_v, pv[:, :, 1, :])

            # Y col = a*128 + z' = (2a + h)*64 + t  (z' = h*64 + t).
            yre_v3 = yre[:].rearrange("p (k t) -> p k t", k=128, t=64)
            yim_v3 = yim[:].rearrange("p (k t) -> p k t", k=128, t=64)

            # Touch each engine's written Y regions (corner of every group
            # region) so P2's stationary reads need no multi-proc waits.
            y_touches = []
            for ten in (yre, yim):
                corner = ten[:].rearrange(
                    "p (gg rest) -> p gg rest", gg=16, rest=512)[:, :, 0]
                y_touches.append(touch(corner))

            # ---- P2 (contract m) + P3 (G3) + SQ + P4, per z'-pair tile t
            for g in range(16):
                ps2 = psA_pool.tile([128, 1024], F32, tag="ps")
                l2re = l2_pool.tile([128, 512], DT, tag="l2re")
                l2im = l2_pool.tile([128, 512], DT, tag="l2im")
                for q in range(4):
                    zp = 4 * g + q
                    mm = nc.tensor.matmul(ps2[:, 256 * q:256 * q + 256],
                                          yre_v3[:, :, zp], G2a[:],
                                          start=True, stop=False)
                    if g == 0 and q == 0:
                        pin(mm, *y_touches)
                    nc.tensor.matmul(ps2[:, 256 * q:256 * q + 256],
                                     yim_v3[:, :, zp], G2b[:],
                                     start=False, stop=True)
                pv2 = ps2[:].rearrange("p (q t m) -> p q t m", q=4, t=2, m=128)
                l2re_v = l2re[:].rearrange("p (q m) -> p q m", q=4, m=128)
                l2im_v = l2im[:].rearrange("p (q m) -> p q m", q=4, m=128)
                if g % 3 == 2:
                    nc.scalar.copy(l2re_v, pv2[:, :, 0, :])
                    nc.scalar.copy(l2im_v, pv2[:, :, 1, :])
                else:
                    nc.vector.tensor_copy(l2re_v, pv2[:, :, 0, :])
                    nc.vector.tensor_copy(l2im_v, pv2[:, :, 1, :])

                ps3 = psB_pool.tile([128, 1024], F32, tag="ps3")
                nc.tensor.matmul(ps3[:, 0:512], G3S[:, 0:128], l2re[:],
                                 start=True, stop=False)
                nc.tensor.matmul(ps3[:, 0:512], G3S[:, 256:384], l2im[:],
                                 start=False, stop=True)
                nc.tensor.matmul(ps3[:, 512:1024], G3S[:, 128:256], l2re[:],
                                 start=True, stop=False)
                nc.tensor.matmul(ps3[:, 512:1024], G3S[:, 0:128], l2im[:],
                                 start=False, stop=True)

                sq = sq_pool.tile([128, 1024], F32R, tag="sq")
                nc.scalar.square(sq[:], ps3[:])

                ps4 = ps4_pool.tile([16, 512], F32, tag="ps4")
                nc.tensor.matmul(ps4[:], R[:], sq[:, 0:512], start=True, stop=False)
                nc.tensor.matmul(ps4[:], R[:], sq[:, 512:1024], start=False, stop=True)
                fsb = f_pool.tile([10, 512], F32, tag="fsb")
                nc.vector.tensor_copy(fsb[:], ps4[0:10, :])
                nc.sync.dma_start(fout[r][:, 512 * g:512 * g + 512], fsb[:])

        # Keep touch outputs live: copy scratch corner out via DVE.
        dbg_sb = const_pool.tile([1, 1], F32)
        nc.vector.tensor_copy(dbg_sb[:], scratch[0:1, 0:1])
        nc.sync.dma_start(dbg[:], dbg_sb[:])
    nc.compile()
    return nc


# ------------------------------------------------------------------ wrapper
_CACHE = {}


def kernel(state, params, W, b):
    if USE_BF16:
        import ml_dtypes
        state = np.ascontiguousarray(np.asarray(state), dtype=ml_dtypes.bfloat16)
    else:
        state = np.ascontiguousarray(np.asarray(state), dtype=np.float32)
    consts, w = _host_precompute(np.asarray(params), np.asarray(W))

    if "nc" not in _CACHE:
        _CACHE["nc"] = build_bass()
    nc = _CACHE["nc"]

    in_maps = []
    for c in range(NCORES):
        m = {"state": state[RPC * c:RPC * (c + 1)]}
        m.update(consts)
        in_maps.append(m)
    res = run_bass_kernel_spmd(nc, in_maps, list(range(NCORES)))
    F = np.concatenate([res.results[c]["F"] for c in range(NCORES)], axis=0)
    out = _host_finish(F, w, np.asarray(b))
    return out.astype(np.float32)



# revision 2
# speedup vs baseline: 1.0634x; 1.0634x over previous
"""Trainium2 Bass kernel for nn_AdvancedRegressionModel (20-qubit circuit regression).

Math: the reference circuit collapses to out_b = sum_j g_j |(A psi_b)_j|^2 + b0
where A = kron of 20 fused 2x2 gates (RY_k @ RX_k per wire) and g absorbs the
CNOT chain (a basis permutation), the <Z_i> measurements, and the linear head
via prefix-sign weights.

Wire blocks: q = wires 0-6 (a:0-5 + m1:6), m2 = wires 7-12, z = wires 13-19.
Per batch row (4 rows/core, batch-sharded over 8 cores):
  load  [q:128, (m2:64, z:128)] fp16 in 16 contiguous chunk DMAs      (DMA)
  P0    contract q: data-stationary matmuls -> PSUM [z, (a | rh,t,h)] (PE)
  drain y [z, (rh, t, k=2*m2+h)] fp16 -- G1's columns are host-permuted
        so P2's stationary picks land CONTIGUOUS in y                 (DVE)
  P2    contract z: stationary = y[:, rh, t, :] (contiguous fp16 ->
        fast weight load), moving G2a/G2b                             (PE)
  drain l2 [k, (rh', t, z')] fp16                                     (DVE)
  P3    contract (m2,h): G3 = kron(Um2, I2) gate-stationary           (PE)
  SQ    ACT square with x512 scale -> fp16 (range-safe)               (ACT)
  P4    reduce with 16-col sign-weight stationary R -> F[10, 8192]    (PE)
  host  finish: out_r = sum_kn F[k,n] * w_k(n) / 512^2 + b

fp16 keeps 10 mantissa bits (same as f32r) -> rel err ~1.4e-3, while halving
DMA/SBUF/LDWEIGHTS cost and freeing matmuls from the f32r self-loading
single-wait constraint (no PE "touch" funnels needed).
"""

import numpy as np
from contextlib import ExitStack

import concourse.mybir as mybir
from concourse import bacc, bass, tile
from concourse.bass_utils import run_bass_kernel_spmd

NW = 20
DIM = 2 ** NW
BATCH = 32
NCORES = 8
RPC = BATCH // NCORES  # rows per core

F32 = mybir.dt.float32
F16 = mybir.dt.float16
SQ_SCALE = 512.0


# ----------------------------------------------------------------- host math
def _gates(params):
    A = []
    for k in range(NW):
        c, s = np.cos(params[k] / 2), np.sin(params[k] / 2)
        RX = np.array([[c, -1j * s], [-1j * s, c]], dtype=np.complex128)
        c2, s2 = np.cos(params[k + NW] / 2), np.sin(params[k + NW] / 2)
        RY = np.array([[c2, -s2], [s2, c2]], dtype=np.complex128)
        A.append(RY @ RX)
    return A


def _kron_list(lst):
    out = lst[0]
    for x in lst[1:]:
        out = np.kron(out, x)
    return out


def _prefix_weights(Wv, wire_lo, wire_hi):
    n = wire_hi - wire_lo + 1
    v = np.arange(2 ** n)
    signs = np.stack([1 - 2 * ((v >> (n - 1 - k)) & 1) for k in range(n)])
    pref = np.cumprod(signs, axis=0)
    g = sum(Wv[wire_lo + i] * pref[i] for i in range(n))
    return g.astype(np.float64), pref[n - 1].astype(np.float64)


def _host_precompute(params, W):
    params = np.asarray(params, dtype=np.float64)
    Wv = np.asarray(W, dtype=np.float64).reshape(-1)
    A = _gates(params)
    Gq = np.kron(_kron_list(A[0:6]), A[6])   # 128x128 on q = (a, m1)
    Uz = _kron_list(A[13:20])                # 128x128 on z
    Um2 = _kron_list(A[7:13])                # 64x64
    G3 = np.kron(Um2, np.eye(2))             # 128x128 on (m2, h)

    # G1 [q:128, nu:256], nu = rh*128 + t*2 + h with q' = h*64 + t
    G1 = np.zeros((128, 256))
    qp = np.arange(128)
    h, t = qp >> 6, qp & 63
    nu_re = t * 2 + h
    G1[:, nu_re] = Gq.real.T
    G1[:, 128 + nu_re] = Gq.imag.T

    G2a = np.concatenate([Uz.real.T, Uz.imag.T], axis=1)    # [z, 256]
    G2b = np.concatenate([-Uz.imag.T, Uz.real.T], axis=1)
    G3S = np.concatenate([G3.real.T, G3.imag.T, -G3.imag.T], axis=1)

    ga, sA = _prefix_weights(Wv, 0, 5)    # [64]
    gm, sM = _prefix_weights(Wv, 6, 12)   # [128]
    gz, _ = _prefix_weights(Wv, 13, 19)   # [128]

    # R [p3:128, 16], p3 = m2'*2 + h
    p = np.arange(128)
    m2p, hh = p >> 1, p & 1
    cols = [(hh == 0).astype(float), (hh == 1).astype(float)]
    for h0 in (0, 1):
        for m10 in (0, 1):
            cols.append((hh == h0) * gm[m10 * 64 + m2p])
    for h0 in (0, 1):
        for m10 in (0, 1):
            cols.append((hh == h0) * sM[m10 * 64 + m2p])
    R = np.stack(cols, axis=1)
    R = np.concatenate([R, np.zeros((128, 6))], axis=1)

    # free weights w[k, n], n = t*128 + z'; a' = h*32 + (t>>1), m1' = t&1
    n = np.arange(8192)
    t_, zp = n >> 7, n & 127
    m1p = t_ & 1
    w = np.zeros((10, 8192))
    w[0] = ga[0 * 32 + (t_ >> 1)]
    w[1] = ga[1 * 32 + (t_ >> 1)]
    k = 2
    for h0 in (0, 1):
        for m10 in (0, 1):
            w[k] = (m1p == m10) * sA[h0 * 32 + (t_ >> 1)]
            k += 1
    for h0 in (0, 1):
        for m10 in (0, 1):
            w[k] = (m1p == m10) * sA[h0 * 32 + (t_ >> 1)] * gz[zp]
            k += 1
    w /= SQ_SCALE ** 2

    consts = {
        "G1": np.ascontiguousarray(G1, dtype=np.float16),
        "G2a": np.ascontiguousarray(G2a, dtype=np.float16),
        "G2b": np.ascontiguousarray(G2b, dtype=np.float16),
        "G3S": np.ascontiguousarray(G3S, dtype=np.float16),
        "R": np.ascontiguousarray(R, dtype=np.float16),
    }
    return consts, w


def _host_finish(F, w, b):
    # F: [B, 10, 8192]
    out = np.einsum("bkn,kn->b", F.astype(np.float64), w)
    return out + float(np.asarray(b).reshape(-1)[0])


# ------------------------------------------------------------- bass program
def build_bass():
    nc = bacc.Bacc("TRN2", target_bir_lowering=False)
    st = nc.declare_dram_parameter("state", [RPC, DIM], F16, isOutput=False)
    g1 = nc.declare_dram_parameter("G1", [128, 256], F16, isOutput=False)
    g2a = nc.declare_dram_parameter("G2a", [128, 256], F16, isOutput=False)
    g2b = nc.declare_dram_parameter("G2b", [128, 256], F16, isOutput=False)
    g3s = nc.declare_dram_parameter("G3S", [128, 384], F16, isOutput=False)
    rw = nc.declare_dram_parameter("R", [128, 16], F16, isOutput=False)
    fout = nc.declare_dram_parameter("F", [RPC, 10, 8192], F32, isOutput=True)

    SQF = mybir.ActivationFunctionType.Square

    with ExitStack() as ctx:
        tc = ctx.enter_context(tile.TileContext(nc))
        const_pool = ctx.enter_context(tc.tile_pool(name="const", bufs=1))
        x0_pool = ctx.enter_context(tc.tile_pool(name="x0", bufs=2))
        y_pool = ctx.enter_context(tc.tile_pool(name="y", bufs=2))
        l2_pool = ctx.enter_context(tc.tile_pool(name="l2", bufs=3))
        sq_pool = ctx.enter_context(tc.tile_pool(name="sq", bufs=3))
        f_pool = ctx.enter_context(tc.tile_pool(name="f", bufs=3))
        psA_pool = ctx.enter_context(
            tc.tile_pool(name="psA", bufs=2, space=bass.MemorySpace.PSUM))
        ps3_pool = ctx.enter_context(
            tc.tile_pool(name="ps3", bufs=2, space=bass.MemorySpace.PSUM))
        ps4_pool = ctx.enter_context(
            tc.tile_pool(name="ps4", bufs=2, space=bass.MemorySpace.PSUM))

        G1 = const_pool.tile([128, 256], F16)
        G2a = const_pool.tile([128, 256], F16)
        G2b = const_pool.tile([128, 256], F16)
        G3S = const_pool.tile([128, 384], F16)
        R = const_pool.tile([128, 16], F16)

        def load_row(r):
            srcv = st[r].rearrange("(q f) -> q f", q=128, f=8192)
            x0s = []
            for c in range(16):
                xt = x0_pool.tile([128, 512], F16, tag=f"x0{c}")
                nc.sync.dma_start(xt[:], srcv[:, 512 * c:512 * c + 512])
                x0s.append(xt)
            return x0s

        preload = {0: load_row(0)}

        nc.sync.dma_start(G1[:], g1[:])
        nc.sync.dma_start(G2a[:], g2a[:])
        nc.sync.dma_start(G2b[:], g2b[:])
        nc.sync.dma_start(G3S[:], g3s[:])
        nc.sync.dma_start(R[:], rw[:])

        for r in range(RPC):
            x0s = preload[r] if r in preload else load_row(r)

            # y [z:128, (rh:2, t:64, k:128)] fp16
            y = y_pool.tile([128, 16384], F16, tag="y")
            y_v = y[:].rearrange("p (rh t k) -> p rh t k", rh=2, t=64, k=128)

            # ---- P0: contract q.  group g handles a = 4g..4g+3 (one chunk)
            for g in range(16):
                ps = psA_pool.tile([128, 1024], F32, tag="ps")
                for q in range(4):
                    nc.tensor.matmul(
                        ps[:, 256 * q:256 * q + 256],
                        x0s[g][:, 128 * q:128 * q + 128], G1[:],
                        start=True, stop=True)
                # drain: ps cols = (a:4, rh:2, t:64, h:2) -> y[,rh,t,8g+2a+h]
                pv = ps[:].rearrange("p (a rh t h) -> p rh t a h",
                                     a=4, rh=2, t=64, h=2)
                for rh in range(2):
                    dst = y_v[:, rh, :, 8 * g:8 * g + 8].rearrange(
                        "p t (a h) -> p t a h", a=4, h=2)
                    nc.vector.tensor_copy(dst, pv[:, rh])

            # ---- P2 (contract z) + P3 (G3) + SQ + P4, per group of 4 t
            for g in range(16):
                ps2 = psA_pool.tile([128, 1024], F32, tag="ps")
                for q in range(4):
                    t = 4 * g + q
                    nc.tensor.matmul(ps2[:, 256 * q:256 * q + 256],
                                     y_v[:, 0, t, :], G2a[:],
                                     start=True, stop=False)
                    nc.tensor.matmul(ps2[:, 256 * q:256 * q + 256],
                                     y_v[:, 1, t, :], G2b[:],
                                     start=False, stop=True)
                # drain: ps2 cols = (t:4, rh':2, z':128) -> l2 (rh', t, z')
                l2 = l2_pool.tile([128, 1024], F16, tag="l2")
                pv2 = ps2[:].rearrange("p (t rh z) -> p rh t z",
                                       t=4, rh=2, z=128)
                l2_v = l2[:].rearrange("p (rh t z) -> p rh t z",
                                       rh=2, t=4, z=128)
                nc.vector.tensor_copy(l2_v[:, 0], pv2[:, 0])
                nc.vector.tensor_copy(l2_v[:, 1], pv2[:, 1])

                psR = ps3_pool.tile([128, 512], F32, tag="ps3")
                psI = ps3_pool.tile([128, 512], F32, tag="ps3")
                nc.tensor.matmul(psR[:], G3S[:, 0:128], l2[:, 0:512],
                                 start=True, stop=False)
                nc.tensor.matmul(psR[:], G3S[:, 256:384], l2[:, 512:1024],
                                 start=False, stop=True)
                nc.tensor.matmul(psI[:], G3S[:, 128:256], l2[:, 0:512],
                                 start=True, stop=False)
                nc.tensor.matmul(psI[:], G3S[:, 0:128], l2[:, 512:1024],
                                 start=False, stop=True)

                sq = sq_pool.tile([128, 1024], F16, tag="sq")
                nc.scalar.activation(sq[:, 0:512], psR[:], SQF,
                                     scale=SQ_SCALE)
                nc.scalar.activation(sq[:, 512:1024], psI[:], SQF,
                                     scale=SQ_SCALE)

                ps4 = ps4_pool.tile([16, 512], F32, tag="ps4")
                nc.tensor.matmul(ps4[:], R[:], sq[:, 0:512],
                                 start=True, stop=False)
                nc.tensor.matmul(ps4[:], R[:], sq[:, 512:1024],
                                 start=False, stop=True)
                fsb = f_pool.tile([10, 512], F32, tag="fsb")
                nc.vector.tensor_copy(fsb[:], ps4[0:10, :])
                nc.sync.dma_start(fout[r][:, 512 * g:512 * g + 512], fsb[:])
    nc.compile()
    return nc


# ------------------------------------------------------------------ wrapper
_CACHE = {}


def kernel(state, params, W, b):
    state = np.ascontiguousarray(np.asarray(state), dtype=np.float16)
    consts, w = _host_precompute(np.asarray(params), np.asarray(W))

    if "nc" not in _CACHE:
        _CACHE["nc"] = build_bass()
    nc = _CACHE["nc"]

    in_maps = []
    for c in range(NCORES):
        m = {"state": state[RPC * c:RPC * (c + 1)]}
        m.update(consts)
        in_maps.append(m)
    res = run_bass_kernel_spmd(nc, in_maps, list(range(NCORES)))
    F = np.concatenate([res.results[c]["F"] for c in range(NCORES)], axis=0)
    out = _host_finish(F, w, np.asarray(b))
    return out.astype(np.float32)


# revision 10
# speedup vs baseline: 1.2037x; 1.1320x over previous
"""Trainium2 Bass kernel for nn_AdvancedRegressionModel (20-qubit circuit regression).

Math: the reference circuit collapses to out_b = sum_j g_j |(A psi_b)_j|^2 + b0
where A = kron of 20 fused 2x2 gates (RY_k @ RX_k per wire) and g absorbs the
CNOT chain (a basis permutation), the <Z_i> measurements, and the linear head
via prefix-sign weights.

Wire blocks: q = wires 0-6 (a:0-5 + m1:6), m2 = wires 7-12, z = wires 13-19.
Per batch row (4 rows/core, batch-sharded over 8 cores):
  load  [q:128, (m2:64, z:128)] fp16 in 16 contiguous chunk DMAs      (DMA)
  P0    contract q: data-stationary matmuls -> PSUM [z, (a | rh,t,h)] (PE)
  drain y [z, (rh, t, k=2*m2+h)] fp16 -- G1's columns are host-permuted
        so P2's stationary picks land CONTIGUOUS in y                 (DVE)
  P2    contract z: stationary = y[:, rh, t, :] (contiguous fp16 ->
        fast weight load), moving G2a/G2b                             (PE)
  drain l2 [k, (rh', t, z')] fp16                                     (DVE)
  P3    contract (m2,h): G3 = kron(Um2, I2) gate-stationary           (PE)
  SQ    ACT square with x512 scale -> fp16 (range-safe)               (ACT)
  P4    reduce with 16-col sign-weight stationary R -> F[10, 8192]    (PE)
  host  finish: out_r = sum_kn F[k,n] * w_k(n) / 512^2 + b

fp16 keeps 10 mantissa bits (same as f32r) -> rel err ~1.4e-3, while halving
DMA/SBUF/LDWEIGHTS cost and freeing matmuls from the f32r self-loading
single-wait constraint (no PE "touch" funnels needed).
"""

import numpy as np
from contextlib import ExitStack

import concourse.mybir as mybir
from concourse import bacc, bass, tile
from concourse.bass_utils import run_bass_kernel_spmd

NW = 20
DIM = 2 ** NW
BATCH = 32
NCORES = 8
RPC = BATCH // NCORES  # rows per core

F32 = mybir.dt.float32
F16 = mybir.dt.float16
SQ_SCALE = 512.0


# ----------------------------------------------------------------- host math
def _gates(params):
    A = []
    for k in range(NW):
        c, s = np.cos(params[k] / 2), np.sin(params[k] / 2)
        RX = np.array([[c, -1j * s], [-1j * s, c]], dtype=np.complex128)
        c2, s2 = np.cos(params[k + NW] / 2), np.sin(params[k + NW] / 2)
        RY = np.array([[c2, -s2], [s2, c2]], dtype=np.complex128)
        A.append(RY @ RX)
    return A


def _kron_list(lst):
    out = lst[0]
    for x in lst[1:]:
        out = np.kron(out, x)
    return out


def _prefix_weights(Wv, wire_lo, wire_hi):
    n = wire_hi - wire_lo + 1
    v = np.arange(2 ** n)
    signs = np.stack([1 - 2 * ((v >> (n - 1 - k)) & 1) for k in range(n)])
    pref = np.cumprod(signs, axis=0)
    g = sum(Wv[wire_lo + i] * pref[i] for i in range(n))
    return g.astype(np.float64), pref[n - 1].astype(np.float64)


def _host_precompute(params, W):
    params = np.asarray(params, dtype=np.float64)
    Wv = np.asarray(W, dtype=np.float64).reshape(-1)
    A = _gates(params)
    Gq = np.kron(_kron_list(A[0:6]), A[6])   # 128x128 on q = (a, m1)
    Uz = _kron_list(A[13:20])                # 128x128 on z
    Um2 = _kron_list(A[7:13])                # 64x64
    G3 = np.kron(Um2, np.eye(2))             # 128x128 on (m2, h)

    # G1 [q:128, nu:256], nu = rh*128 + t*2 + h with q' = h*64 + t
    G1 = np.zeros((128, 256))
    qp = np.arange(128)
    h, t = qp >> 6, qp & 63
    nu_re = t * 2 + h
    G1[:, nu_re] = Gq.real.T
    G1[:, 128 + nu_re] = Gq.imag.T

    G2a = np.concatenate([Uz.real.T, Uz.imag.T], axis=1)    # [z, 256]
    G2b = np.concatenate([-Uz.imag.T, Uz.real.T], axis=1)
    # SQ_SCALE folded into G3S so squares need no ACT scale param
    G3S = SQ_SCALE * np.concatenate(
        [G3.real.T, G3.imag.T, -G3.imag.T], axis=1)

    ga, sA = _prefix_weights(Wv, 0, 5)    # [64]
    gm, sM = _prefix_weights(Wv, 6, 12)   # [128]
    gz, _ = _prefix_weights(Wv, 13, 19)   # [128]

    # R [p3:128, 16], p3 = m2'*2 + h
    p = np.arange(128)
    m2p, hh = p >> 1, p & 1
    cols = [(hh == 0).astype(float), (hh == 1).astype(float)]
    for h0 in (0, 1):
        for m10 in (0, 1):
            cols.append((hh == h0) * gm[m10 * 64 + m2p])
    for h0 in (0, 1):
        for m10 in (0, 1):
            cols.append((hh == h0) * sM[m10 * 64 + m2p])
    R = np.stack(cols, axis=1)
    R = np.concatenate([R, np.zeros((128, 6))], axis=1)

    # free weights w[k, n], n = t*128 + z'; a' = h*32 + (t>>1), m1' = t&1
    n = np.arange(8192)
    t_, zp = n >> 7, n & 127
    m1p = t_ & 1
    w = np.zeros((10, 8192))
    w[0] = ga[0 * 32 + (t_ >> 1)]
    w[1] = ga[1 * 32 + (t_ >> 1)]
    k = 2
    for h0 in (0, 1):
        for m10 in (0, 1):
            w[k] = (m1p == m10) * sA[h0 * 32 + (t_ >> 1)]
            k += 1
    for h0 in (0, 1):
        for m10 in (0, 1):
            w[k] = (m1p == m10) * sA[h0 * 32 + (t_ >> 1)] * gz[zp]
            k += 1
    w /= SQ_SCALE ** 2

    consts = {
        "G1": np.ascontiguousarray(G1, dtype=np.float16),
        "G2a": np.ascontiguousarray(G2a, dtype=np.float16),
        "G2b": np.ascontiguousarray(G2b, dtype=np.float16),
        "G3S": np.ascontiguousarray(G3S, dtype=np.float16),
        "R": np.ascontiguousarray(R, dtype=np.float16),
    }
    return consts, w


def _host_finish(F, w, b):
    # F: [B, 10, 8192]
    out = np.einsum("bkn,kn->b", F.astype(np.float64), w)
    return out + float(np.asarray(b).reshape(-1)[0])


# ------------------------------------------------------------- bass program
def build_bass():
    nc = bacc.Bacc("TRN2", target_bir_lowering=False)
    st = nc.declare_dram_parameter("state", [RPC, DIM], F16, isOutput=False)
    g1 = nc.declare_dram_parameter("G1", [128, 256], F16, isOutput=False)
    g2a = nc.declare_dram_parameter("G2a", [128, 256], F16, isOutput=False)
    g2b = nc.declare_dram_parameter("G2b", [128, 256], F16, isOutput=False)
    g3s = nc.declare_dram_parameter("G3S", [128, 384], F16, isOutput=False)
    rw = nc.declare_dram_parameter("R", [128, 16], F16, isOutput=False)
    fout = nc.declare_dram_parameter("F", [RPC, 10, 8192], F32, isOutput=True)

    with ExitStack() as ctx:
        tc = ctx.enter_context(tile.TileContext(nc))
        const_pool = ctx.enter_context(tc.tile_pool(name="const", bufs=1))
        x0_pool = ctx.enter_context(tc.tile_pool(name="x0", bufs=2))
        y_pool = ctx.enter_context(tc.tile_pool(name="y", bufs=2))
        l2_pool = ctx.enter_context(tc.tile_pool(name="l2", bufs=3))
        sq_pool = ctx.enter_context(tc.tile_pool(name="sq", bufs=3))
        f_pool = ctx.enter_context(tc.tile_pool(name="f", bufs=3))
        psA_pool = ctx.enter_context(
            tc.tile_pool(name="psA", bufs=2, space=bass.MemorySpace.PSUM))
        ps3_pool = ctx.enter_context(
            tc.tile_pool(name="ps3", bufs=2, space=bass.MemorySpace.PSUM))
        ps4_pool = ctx.enter_context(
            tc.tile_pool(name="ps4", bufs=2, space=bass.MemorySpace.PSUM))

        G1 = const_pool.tile([128, 256], F16)
        G2a = const_pool.tile([128, 256], F16)
        G2b = const_pool.tile([128, 256], F16)
        G3S = const_pool.tile([128, 384], F16)
        R = const_pool.tile([128, 16], F16)

        def load_row(r):
            srcv = st[r].rearrange("(q f) -> q f", q=128, f=8192)
            x0s = []
            for c in range(16):
                xt = x0_pool.tile([128, 512], F16, tag=f"x0{c}")
                nc.sync.dma_start(xt[:], srcv[:, 512 * c:512 * c + 512])
                x0s.append(xt)
            return x0s

        nc.sync.dma_start(G1[:], g1[:])
        nc.sync.dma_start(G2a[:], g2a[:])
        nc.sync.dma_start(G2b[:], g2b[:])
        nc.sync.dma_start(G3S[:], g3s[:])
        nc.sync.dma_start(R[:], rw[:])

        preload = {0: load_row(0)}

        for r in range(RPC):
            x0s = preload[r] if r in preload else load_row(r)

            # y [z:128, (rh:2, t:64, k:128)] fp16
            y = y_pool.tile([128, 16384], F16, tag="y")
            y_v = y[:].rearrange("p (rh t k) -> p rh t k", rh=2, t=64, k=128)

            # ---- P0: contract q.  group g handles a = 4g..4g+3 (one chunk)
            for g in range(16):
                ps = psA_pool.tile([128, 1024], F32, tag="ps")
                for q in range(4):
                    nc.tensor.matmul(
                        ps[:, 256 * q:256 * q + 256],
                        x0s[g][:, 128 * q:128 * q + 128], G1[:],
                        start=True, stop=True)
                # drain: ps cols = (a:4, rh:2, t:64, h:2) -> y[,rh,t,8g+2a+h]
                pv = ps[:].rearrange("p (a rh t h) -> p rh t a h",
                                     a=4, rh=2, t=64, h=2)
                for rh in range(2):
                    dst = y_v[:, rh, :, 8 * g:8 * g + 8].rearrange(
                        "p t (a h) -> p t a h", a=4, h=2)
                    if rh == 0:
                        nc.vector.tensor_copy(dst, pv[:, rh])
                    else:
                        nc.scalar.copy(dst, pv[:, rh])

            # ---- P2 (contract z) + P3 (G3) + SQ + P4, per group of 4 t
            for g in range(16):
                ps2 = psA_pool.tile([128, 1024], F32, tag="ps")
                for q in range(4):
                    t = 4 * g + q
                    nc.tensor.matmul(ps2[:, 256 * q:256 * q + 256],
                                     y_v[:, 0, t, :], G2a[:],
                                     start=True, stop=False)
                    nc.tensor.matmul(ps2[:, 256 * q:256 * q + 256],
                                     y_v[:, 1, t, :], G2b[:],
                                     start=False, stop=True)
                # drain: ps2 cols = (t:4, rh':2, z':128) -> l2 (rh', t, z')
                l2 = l2_pool.tile([128, 1024], F16, tag="l2")
                pv2 = ps2[:].rearrange("p (t rh z) -> p rh t z",
                                       t=4, rh=2, z=128)
                l2_v = l2[:].rearrange("p (rh t z) -> p rh t z",
                                       rh=2, t=4, z=128)
                nc.vector.tensor_copy(l2_v, pv2)

                psR = ps3_pool.tile([128, 512], F32, tag="ps3")
                psI = ps3_pool.tile([128, 512], F32, tag="ps3")
                nc.tensor.matmul(psR[:], G3S[:, 0:128], l2[:, 0:512],
                                 start=True, stop=False)
                nc.tensor.matmul(psR[:], G3S[:, 256:384], l2[:, 512:1024],
                                 start=False, stop=True)
                nc.tensor.matmul(psI[:], G3S[:, 128:256], l2[:, 0:512],
                                 start=True, stop=False)
                nc.tensor.matmul(psI[:], G3S[:, 0:128], l2[:, 512:1024],
                                 start=False, stop=True)

                sq = sq_pool.tile([128, 1024], F16, tag="sq")
                nc.scalar.square(sq[:, 0:512], psR[:])
                nc.scalar.square(sq[:, 512:1024], psI[:])

                ps4 = ps4_pool.tile([16, 512], F32, tag="ps4")
                nc.tensor.matmul(ps4[:], R[:], sq[:, 0:512],
                                 start=True, stop=False)
                nc.tensor.matmul(ps4[:], R[:], sq[:, 512:1024],
                                 start=False, stop=True)
                fsb = f_pool.tile([10, 512], F32, tag="fsb")
                if g % 2 == 0:
                    nc.vector.tensor_copy(fsb[:], ps4[0:10, :])
                else:
                    nc.scalar.copy(fsb[:], ps4[0:10, :])
                nc.sync.dma_start(fout[r][:, 512 * g:512 * g + 512], fsb[:])
    nc.compile()
    return nc


# ------------------------------------------------------------------ wrapper
_CACHE = {}


def kernel(state, params, W, b):
    state = np.ascontiguousarray(np.asarray(state), dtype=np.float16)
    consts, w = _host_precompute(np.asarray(params), np.asarray(W))

    if "nc" not in _CACHE:
        _CACHE["nc"] = build_bass()
    nc = _CACHE["nc"]

    in_maps = []
    for c in range(NCORES):
        m = {"state": state[RPC * c:RPC * (c + 1)]}
        m.update(consts)
        in_maps.append(m)
    res = run_bass_kernel_spmd(nc, in_maps, list(range(NCORES)))
    F = np.concatenate([res.results[c]["F"] for c in range(NCORES)], axis=0)
    out = _host_finish(F, w, np.asarray(b))
    return out.astype(np.float32)


# revision 14
# speedup vs baseline: 1.2164x; 1.0105x over previous
"""Trainium2 Bass kernel for nn_AdvancedRegressionModel (20-qubit circuit regression).

Math: the reference circuit collapses to out_b = sum_j g_j |(A psi_b)_j|^2 + b0
where A = kron of 20 fused 2x2 gates (RY_k @ RX_k per wire) and g absorbs the
CNOT chain (a basis permutation), the <Z_i> measurements, and the linear head
via prefix-sign weights.

Wire blocks: q = wires 0-6 (a:0-5 + m1:6), m2 = wires 7-12, z = wires 13-19.
Per batch row (4 rows/core, batch-sharded over 8 cores):
  load  [q:128, (m2:64, z:128)] fp16 in 16 contiguous chunk DMAs      (DMA)
  P0    contract q: data-stationary matmuls -> PSUM [z, (a | rh,t,h)] (PE)
  drain y [z, (rh, t, k=2*m2+h)] fp16 -- G1's columns are host-permuted
        so P2's stationary picks land CONTIGUOUS in y                 (DVE)
  P2    contract z: stationary = y[:, rh, t, :] (contiguous fp16 ->
        fast weight load), moving G2a/G2b                             (PE)
  drain l2 [k, (rh', t, z')] fp16                                     (DVE)
  P3    contract (m2,h): G3 = kron(Um2, I2) gate-stationary           (PE)
  SQ    ACT square with x512 scale -> fp16 (range-safe)               (ACT)
  P4    reduce with 16-col sign-weight stationary R -> F[10, 8192]    (PE)
  host  finish: out_r = sum_kn F[k,n] * w_k(n) / 512^2 + b

fp16 keeps 10 mantissa bits (same as f32r) -> rel err ~1.4e-3, while halving
DMA/SBUF/LDWEIGHTS cost and freeing matmuls from the f32r self-loading
single-wait constraint (no PE "touch" funnels needed).
"""

import numpy as np
from contextlib import ExitStack

import concourse.mybir as mybir
from concourse import bacc, bass, tile
from concourse.bass_utils import run_bass_kernel_spmd

NW = 20
DIM = 2 ** NW
BATCH = 32
NCORES = 8
RPC = BATCH // NCORES  # rows per core

F32 = mybir.dt.float32
F16 = mybir.dt.float16
SQ_SCALE = 512.0


# ----------------------------------------------------------------- host math
def _gates(params):
    A = []
    for k in range(NW):
        c, s = np.cos(params[k] / 2), np.sin(params[k] / 2)
        RX = np.array([[c, -1j * s], [-1j * s, c]], dtype=np.complex128)
        c2, s2 = np.cos(params[k + NW] / 2), np.sin(params[k + NW] / 2)
        RY = np.array([[c2, -s2], [s2, c2]], dtype=np.complex128)
        A.append(RY @ RX)
    return A


def _kron_list(lst):
    out = lst[0]
    for x in lst[1:]:
        out = np.kron(out, x)
    return out


def _prefix_weights(Wv, wire_lo, wire_hi):
    n = wire_hi - wire_lo + 1
    v = np.arange(2 ** n)
    signs = np.stack([1 - 2 * ((v >> (n - 1 - k)) & 1) for k in range(n)])
    pref = np.cumprod(signs, axis=0)
    g = sum(Wv[wire_lo + i] * pref[i] for i in range(n))
    return g.astype(np.float64), pref[n - 1].astype(np.float64)


def _host_precompute(params, W):
    params = np.asarray(params, dtype=np.float64)
    Wv = np.asarray(W, dtype=np.float64).reshape(-1)
    A = _gates(params)
    Gq = np.kron(_kron_list(A[0:6]), A[6])   # 128x128 on q = (a, m1)
    Uz = _kron_list(A[13:20])                # 128x128 on z
    Um2 = _kron_list(A[7:13])                # 64x64
    G3 = np.kron(Um2, np.eye(2))             # 128x128 on (m2, h)

    # G1 [q:128, nu:256], nu = rh*128 + t*2 + h with q' = h*64 + t
    G1 = np.zeros((128, 256))
    qp = np.arange(128)
    h, t = qp >> 6, qp & 63
    nu_re = t * 2 + h
    G1[:, nu_re] = Gq.real.T
    G1[:, 128 + nu_re] = Gq.imag.T

    G2a = np.concatenate([Uz.real.T, Uz.imag.T], axis=1)    # [z, 256]
    G2b = np.concatenate([-Uz.imag.T, Uz.real.T], axis=1)
    # SQ_SCALE folded into G3S so squares need no ACT scale param
    G3S = SQ_SCALE * np.concatenate(
        [G3.real.T, G3.imag.T, -G3.imag.T], axis=1)

    ga, sA = _prefix_weights(Wv, 0, 5)    # [64]
    gm, sM = _prefix_weights(Wv, 6, 12)   # [128]
    gz, _ = _prefix_weights(Wv, 13, 19)   # [128]

    # R [p3:128, 16], p3 = m2'*2 + h
    p = np.arange(128)
    m2p, hh = p >> 1, p & 1
    cols = [(hh == 0).astype(float), (hh == 1).astype(float)]
    for h0 in (0, 1):
        for m10 in (0, 1):
            cols.append((hh == h0) * gm[m10 * 64 + m2p])
    for h0 in (0, 1):
        for m10 in (0, 1):
            cols.append((hh == h0) * sM[m10 * 64 + m2p])
    R = np.stack(cols, axis=1)
    R = np.concatenate([R, np.zeros((128, 6))], axis=1)

    # free weights w[k, n], n = t*128 + z'; a' = h*32 + (t>>1), m1' = t&1
    n = np.arange(8192)
    t_, zp = n >> 7, n & 127
    m1p = t_ & 1
    w = np.zeros((10, 8192))
    w[0] = ga[0 * 32 + (t_ >> 1)]
    w[1] = ga[1 * 32 + (t_ >> 1)]
    k = 2
    for h0 in (0, 1):
        for m10 in (0, 1):
            w[k] = (m1p == m10) * sA[h0 * 32 + (t_ >> 1)]
            k += 1
    for h0 in (0, 1):
        for m10 in (0, 1):
            w[k] = (m1p == m10) * sA[h0 * 32 + (t_ >> 1)] * gz[zp]
            k += 1
    w /= SQ_SCALE ** 2

    consts = {
        "G1": np.ascontiguousarray(G1, dtype=np.float16),
        "G2a": np.ascontiguousarray(G2a, dtype=np.float16),
        "G2b": np.ascontiguousarray(G2b, dtype=np.float16),
        "G3S": np.ascontiguousarray(G3S, dtype=np.float16),
        "R": np.ascontiguousarray(R, dtype=np.float16),
    }
    return consts, w


def _host_finish(F, w, b):
    # F: [B, 10, 8192]
    out = np.einsum("bkn,kn->b", F.astype(np.float64), w)
    return out + float(np.asarray(b).reshape(-1)[0])


# ------------------------------------------------------------- bass program
def build_bass():
    nc = bacc.Bacc("TRN2", target_bir_lowering=False)
    st = nc.declare_dram_parameter("state", [RPC, DIM], F16, isOutput=False)
    g1 = nc.declare_dram_parameter("G1", [128, 256], F16, isOutput=False)
    g2a = nc.declare_dram_parameter("G2a", [128, 256], F16, isOutput=False)
    g2b = nc.declare_dram_parameter("G2b", [128, 256], F16, isOutput=False)
    g3s = nc.declare_dram_parameter("G3S", [128, 384], F16, isOutput=False)
    rw = nc.declare_dram_parameter("R", [128, 16], F16, isOutput=False)
    fout = nc.declare_dram_parameter("F", [RPC, 10, 8192], F32, isOutput=True)

    with ExitStack() as ctx:
        tc = ctx.enter_context(tile.TileContext(nc))
        const_pool = ctx.enter_context(tc.tile_pool(name="const", bufs=1))
        x0_pool = ctx.enter_context(tc.tile_pool(name="x0", bufs=2))
        y_pool = ctx.enter_context(tc.tile_pool(name="y", bufs=2))
        l2_pool = ctx.enter_context(tc.tile_pool(name="l2", bufs=3))
        sq_pool = ctx.enter_context(tc.tile_pool(name="sq", bufs=3))
        f_pool = ctx.enter_context(tc.tile_pool(name="f", bufs=3))
        psA_pool = ctx.enter_context(
            tc.tile_pool(name="psA", bufs=2, space=bass.MemorySpace.PSUM))
        psB_pool = ctx.enter_context(
            tc.tile_pool(name="psB", bufs=4, space=bass.MemorySpace.PSUM))

        G1 = const_pool.tile([128, 256], F16)
        G2a = const_pool.tile([128, 256], F16)
        G2b = const_pool.tile([128, 256], F16)
        G3S = const_pool.tile([128, 384], F16)
        R = const_pool.tile([128, 16], F16)

        def load_row(r):
            srcv = st[r].rearrange("(q f) -> q f", q=128, f=8192)
            x0s = []
            for c in range(16):
                xt = x0_pool.tile([128, 512], F16, tag=f"x0{c}")
                nc.sync.dma_start(xt[:], srcv[:, 512 * c:512 * c + 512])
                x0s.append(xt)
            return x0s

        # G1 first (needed immediately), then row-0 chunks claim the DMA
        # queues, then the consts needed only from the P2 phase onward.
        nc.sync.dma_start(G1[:], g1[:])
        preload = {0: load_row(0)}
        nc.sync.dma_start(G2a[:], g2a[:])
        nc.sync.dma_start(G2b[:], g2b[:])
        nc.sync.dma_start(G3S[:], g3s[:])
        nc.sync.dma_start(R[:], rw[:])

        for r in range(RPC):
            x0s = preload[r] if r in preload else load_row(r)

            # y [z:128, (rh:2, t:64, k:128)] fp16
            y = y_pool.tile([128, 16384], F16, tag="y")
            y_v = y[:].rearrange("p (rh t k) -> p rh t k", rh=2, t=64, k=128)

            # ---- P0: contract q.  group g handles a = 4g..4g+3 (one chunk)
            for g in range(16):
                ps = psA_pool.tile([128, 1024], F32, tag="ps")
                for q in range(4):
                    nc.tensor.matmul(
                        ps[:, 256 * q:256 * q + 256],
                        x0s[g][:, 128 * q:128 * q + 128], G1[:],
                        start=True, stop=True)
                # drain: ps cols = (a:4, rh:2, t:64, h:2) -> y[,rh,t,8g+2a+h]
                pv = ps[:].rearrange("p (a rh t h) -> p rh t a h",
                                     a=4, rh=2, t=64, h=2)
                for rh in range(2):
                    dst = y_v[:, rh, :, 8 * g:8 * g + 8].rearrange(
                        "p t (a h) -> p t a h", a=4, h=2)
                    if rh == 0:
                        nc.vector.tensor_copy(dst, pv[:, rh])
                    else:
                        nc.scalar.copy(dst, pv[:, rh])

            # ---- P2 (contract z) + P3 (G3) + SQ + P4, per group of 4 t
            for g in range(16):
                ps2 = psA_pool.tile([128, 1024], F32, tag="ps")
                for q in range(4):
                    t = 4 * g + q
                    nc.tensor.matmul(ps2[:, 256 * q:256 * q + 256],
                                     y_v[:, 0, t, :], G2a[:],
                                     start=True, stop=False)
                    nc.tensor.matmul(ps2[:, 256 * q:256 * q + 256],
                                     y_v[:, 1, t, :], G2b[:],
                                     start=False, stop=True)
                # drain: ps2 cols = (t:4, rh':2, z':128) -> l2 (rh', t, z')
                l2 = l2_pool.tile([128, 1024], F16, tag="l2")
                pv2 = ps2[:].rearrange("p (t rh z) -> p rh t z",
                                       t=4, rh=2, z=128)
                l2_v = l2[:].rearrange("p (rh t z) -> p rh t z",
                                       rh=2, t=4, z=128)
                nc.vector.tensor_copy(l2_v, pv2)

                psR = psB_pool.tile([128, 512], F32, tag="psB")
                psI = psB_pool.tile([128, 512], F32, tag="psB")
                nc.tensor.matmul(psR[:], G3S[:, 0:128], l2[:, 0:512],
                                 start=True, stop=False)
                nc.tensor.matmul(psR[:], G3S[:, 256:384], l2[:, 512:1024],
                                 start=False, stop=True)
                nc.tensor.matmul(psI[:], G3S[:, 128:256], l2[:, 0:512],
                                 start=True, stop=False)
                nc.tensor.matmul(psI[:], G3S[:, 0:128], l2[:, 512:1024],
                                 start=False, stop=True)

                sq = sq_pool.tile([128, 1024], F16, tag="sq")
                nc.scalar.square(sq[:, 0:512], psR[:])
                nc.scalar.square(sq[:, 512:1024], psI[:])

                ps4 = psB_pool.tile([16, 512], F32, tag="psB")
                nc.tensor.matmul(ps4[:], R[:], sq[:, 0:512],
                                 start=True, stop=False)
                nc.tensor.matmul(ps4[:], R[:], sq[:, 512:1024],
                                 start=False, stop=True)
                fsb = f_pool.tile([10, 512], F32, tag="fsb")
                if g % 2 == 0:
                    nc.vector.tensor_copy(fsb[:], ps4[0:10, :])
                else:
                    nc.scalar.copy(fsb[:], ps4[0:10, :])
                nc.sync.dma_start(fout[r][:, 512 * g:512 * g + 512], fsb[:])
    nc.compile()
    return nc


# ------------------------------------------------------------------ wrapper
_CACHE = {}


def kernel(state, params, W, b):
    state = np.ascontiguousarray(np.asarray(state), dtype=np.float16)
    consts, w = _host_precompute(np.asarray(params), np.asarray(W))

    if "nc" not in _CACHE:
        _CACHE["nc"] = build_bass()
    nc = _CACHE["nc"]

    in_maps = []
    for c in range(NCORES):
        m = {"state": state[RPC * c:RPC * (c + 1)]}
        m.update(consts)
        in_maps.append(m)
    res = run_bass_kernel_spmd(nc, in_maps, list(range(NCORES)))
    F = np.concatenate([res.results[c]["F"] for c in range(NCORES)], axis=0)
    out = _host_finish(F, w, np.asarray(b))
    return out.astype(np.float32)
